# revision 1
# baseline (speedup 1.0000x reference)
"""DCRNN diffusion-conv GNN forward on 8 trn2 NeuronCores.

Math (reference has H0=0, so the r-gate is dead and every dconv input is x):
  deg_out[v] = sum_{e:src=v} w[e]; deg_in[v] = sum_{e:dst=v} w[e]
  x_o = x / deg_out ; x_i = x / deg_in            (per-row scale)
  T_o1[d] = sum_{e:dst=d} x_o[src[e]]             (pure segment sums, coef
  T_i1[s] = sum_{e:src=s} x_i[dst[e]]              folded into the tables)
  T_o2 = segsum(T_o1/deg_out), T_i2 = segsum(T_i1/deg_in)
  G_g = x@(Wg[0,0]+Wg[1,0])[:32] + T_o1@Wg[0,1][:32] + T_i1@Wg[1,1][:32]
        + T_o2@Wg[0,2][:32] + T_i2@Wg[1,2][:32] + b_g      for g in {z,h}
  out = relu(sigmoid(-G_z) * tanh(G_h)) @ lin_w + lin_b

Distribution: nodes sharded 8 ways; edge lists partitioned by scatter-side
shard; gathers read replicated HBM tables (bf16, quad-row 256B descriptors);
segment sums are PE matmuls with DVE-built one-hot slot->node matrices into a
PSUM-resident [128, W*32] shard accumulator; shard T1 tables are exchanged
with AllGather. One SPMD program: per-(window,parity) slot budgets are maxed
across cores so the instruction stream is core-independent.
"""

import sys

sys.path.insert(0, "/opt/trn_rl_repo")

import numpy as np

N = 100000
C = 32
NCORES = 8
GATE = 64
OUTC = 32
CALL = 1024  # slots per dma_gather (2048+ risks SWDGE ring deadlock, 4096 confirmed hangs)


def _wrap_idx(a):
    # dma_gather index layout: idx i lives at partition i%16, col i//16,
    # replicated to all eight 16-partition groups.
    s = a.shape[0]
    w = a.reshape(s // 16, 16).T.astype(np.int16)
    return np.tile(w, (8, 1))


def _prep_dir(gnode, snode, wval, npad, sh):
    """Homogenized slot/chunk structure for one propagate direction.

    gnode: gather-side node per edge (global), snode: scatter-side node
    (global, defines core = snode//sh), wval: edge weight.
    Returns per-core device arrays + core-independent chunk metadata.
    """
    W = sh // 128
    core = snode // sh
    nl = snode - core * sh
    q = gnode % 4
    w = nl // 128
    cnt = np.zeros((NCORES, W, 4), np.int64)
    np.add.at(cnt, (core, w, q), 1)
    bud = 128 * np.ceil(cnt.max(axis=0) / 128).astype(np.int64)  # [W, 4]
    starts = np.concatenate([[0], np.cumsum(bud.reshape(-1))])[:-1].reshape(W, 4)
    S = int(bud.sum())
    S_pad = ((S + CALL - 1) // CALL) * CALL
    NV = S_pad // 128

    # slot position of each edge: starts[w,q] + rank within (core,w,q)
    key = (core * W + w) * 4 + q
    order = np.argsort(key, kind="stable")
    ranks = np.empty(len(key), np.int64)
    sk = key[order]
    brk = np.concatenate([[0], np.nonzero(np.diff(sk))[0] + 1])
    grp = np.zeros(len(sk), np.int64)
    grp[brk] = brk
    grp = np.maximum.accumulate(grp)
    ranks[order] = np.arange(len(sk)) - grp
    pos = starts[w, q] + ranks

    gidx_all, ldst_all, wgt_all = [], [], []
    for c in range(NCORES):
        m = core == c
        gidx = np.zeros(S_pad, np.int64)
        ldst = np.full(S_pad, -1.0, np.float32)
        wgt = np.zeros(S_pad, np.float32)
        gidx[pos[m]] = gnode[m] // 4
        ldst[pos[m]] = (nl[m] % 128).astype(np.float32)
        wgt[pos[m]] = wval[m]
        gidx_all.append(_wrap_idx(gidx))
        ldst_all.append(ldst.reshape(NV, 128).T.copy())
        wgt_all.append(wgt.reshape(NV, 128).T.copy())

    # chunk metadata (identical for every core)
    chunks = []  # (chunk_index, window, rhs_off, start, stop)
    for wi in range(W):
        cell = [(qi, k) for qi in range(4) for k in range(int(bud[wi, qi]) // 128)]
        for j, (qi, k) in enumerate(cell):
            ch = int(starts[wi, qi]) // 128 + k
            chunks.append((ch, wi, qi * 32, j == 0, j == len(cell) - 1))
    chunks.sort()
    return dict(
        S=S_pad, NV=NV, chunks=chunks,
        gidx=gidx_all, ldst=ldst_all, wgt=wgt_all,
    )


def _host_prep(x, edge_index, edge_weight):
    npad = ((N + 1024 * NCORES - 1) // (1024 * NCORES)) * 1024 * NCORES
    sh = npad // NCORES
    src = edge_index[0].astype(np.int64)
    dst = edge_index[1].astype(np.int64)
    wv = edge_weight.astype(np.float32)
    fwd = _prep_dir(src, dst, wv, npad, sh)  # scatter by dst, gather src
    rev = _prep_dir(dst, src, wv, npad, sh)  # scatter by src, gather dst
    x_pad = np.zeros((npad, C), np.float32)
    x_pad[:N] = x
    return npad, sh, fwd, rev, x_pad


def _build(npad, sh, fwd, rev, stop_after=None):
    import concourse.bacc as bacc
    import concourse.bass as bass
    import concourse.mybir as mybir
    import concourse.tile as tile

    W = sh // 128
    f32 = mybir.dt.float32
    bf16 = mybir.dt.bfloat16
    i16 = mybir.dt.int16
    AF = mybir.ActivationFunctionType
    OP = mybir.AluOpType
    RG = [list(range(NCORES))]

    nc = bacc.Bacc(target_bir_lowering=False)

    # ---------------- parameters ----------------
    x_rm = nc.declare_dram_parameter("x_rm", [npad, C], f32, isOutput=False)
    xT = nc.declare_dram_parameter("xT", [C, sh], f32, isOutput=False)
    io_bf = nc.declare_dram_parameter("io_bf", [128, 128], bf16, isOutput=False)
    id32 = nc.declare_dram_parameter("id32", [128, 128], f32, isOutput=False)
    wstk = nc.declare_dram_parameter("wstk", [6, 32, 128], f32, isOutput=False)
    bcat = nc.declare_dram_parameter("bcat", [128, 1], f32, isOutput=False)
    linw = nc.declare_dram_parameter("linw", [GATE, OUTC], f32, isOutput=False)
    linb = nc.declare_dram_parameter("linb", [OUTC, 1], f32, isOutput=False)
    pin = {}
    for nm, d in (("f", fwd), ("r", rev)):
        pin[nm + "idx"] = nc.declare_dram_parameter(f"{nm}idx", [128, d["S"] // 16], i16, isOutput=False)
        pin[nm + "ldst"] = nc.declare_dram_parameter(f"{nm}ldst", [128, d["NV"]], f32, isOutput=False)
        pin[nm + "wgt"] = nc.declare_dram_parameter(f"{nm}wgt", [128, d["NV"]], f32, isOutput=False)
    outT = nc.declare_dram_parameter("outT", [C, sh], f32, isOutput=True)

    # ---------------- internal DRAM ----------------
    def dram(name, shape, dt):
        return nc.dram_tensor(name, shape, dt)

    rin_flat = dram("rin_flat", [sh], f32)
    rout_flat = dram("rout_flat", [sh], f32)
    rin_full = dram("rin_full", [NCORES * sh], f32)
    rout_full = dram("rout_full", [NCORES * sh], f32)
    rpw_in = dram("rpw_in", [128, W], f32)
    rpw_out = dram("rpw_out", [128, W], f32)
    xo_tab = dram("xo_tab", [npad, C], bf16)
    xi_tab = dram("xi_tab", [npad, C], bf16)
    t1o_b = dram("t1o_b", [sh, C], bf16)
    t1i_b = dram("t1i_b", [sh, C], bf16)
    t1o_tab = dram("t1o_tab", [npad, C], bf16)
    t1i_tab = dram("t1i_tab", [npad, C], bf16)
    to1_raw = dram("to1_raw", [128, W * C], f32)
    ti1_raw = dram("ti1_raw", [128, W * C], f32)
    to2_raw = dram("to2_raw", [128, W * C], f32)

    TC = tile.TileContext

    # -------- helper: deg reduce pass (streamed weights, no gather) --------
    def deg_pass(tc, pool, spool, psum, d, wgt_par, ldst_par, iota):
        wbf = spool.tile([128, d["NV"]], bf16, tag="deg_wbf")
        wsb = spool.tile([128, d["NV"]], f32, tag="deg_wsb")
        lsb = spool.tile([128, d["NV"]], f32, tag="deg_lsb")
        nc.sync.dma_start(out=wsb[:], in_=wgt_par[:])
        nc.sync.dma_start(out=lsb[:], in_=ldst_par[:])
        nc.vector.tensor_copy(out=wbf[:], in_=wsb[:])
        dps = psum.tile([128, W], f32, space="PSUM")
        for ch, wi, off, st, sp in d["chunks"]:
            oh = pool.tile([128, 128], bf16, tag="deg_oh")
            nc.vector.tensor_scalar(
                out=oh[:], in0=iota[:], scalar1=lsb[:, ch : ch + 1],
                scalar2=None, op0=OP.is_equal,
            )
            nc.tensor.matmul(
                dps[:, wi : wi + 1], lhsT=oh[:], rhs=wbf[:, ch : ch + 1],
                start=st, stop=sp,
            )
        return dps, lsb

    # -------- helper: recip + row-major export --------
    def recip_export(tc, pool, psum, dps, rpw_dram, rflat_dram, ident):
        rec = pool.tile([128, W], f32, tag="rec")
        nc.vector.tensor_scalar(
            out=rec[:], in0=dps[:], scalar1=1e-20, scalar2=None, op0=OP.max,
        )
        nc.vector.reciprocal(out=rec[:], in_=rec[:])
        nc.sync.dma_start(out=rpw_dram[:], in_=rec[:])
        tp = psum.tile([W, 128], f32, space="PSUM")
        nc.tensor.transpose(out=tp[:], in_=rec[:], identity=ident[:])
        rm = pool.tile([W, 128], f32, tag="rm")
        nc.scalar.activation(out=rm[:], in_=tp[:], func=AF.Copy)
        nc.sync.dma_start(out=rflat_dram.rearrange("(w p) -> w p", p=128)[:], in_=rm[:])

    # -------- helper: scaled-table build (x * recip -> bf16 table) --------
    def build_table(tc, pool, rfull, tab):
        xv = x_rm.rearrange("(c p w) d -> c p (w d)", p=128, w=W)
        rv = rfull.rearrange("(c p w) -> c p w", p=128, w=W)
        tv = tab.rearrange("(c p w) d -> c p (w d)", p=128, w=W)
        for cc in range(NCORES):
            xt = pool.tile([128, W * C], f32, tag="xs_x")
            rt = pool.tile([128, W], f32, tag="xs_r")
            ot = pool.tile([128, W * C], bf16, tag="xs_o")
            nc.sync.dma_start(out=xt[:], in_=xv[cc])
            nc.sync.dma_start(out=rt[:], in_=rv[cc])
            nc.vector.tensor_tensor(
                out=ot[:].rearrange("p (w d) -> p w d", w=W),
                in0=xt[:].rearrange("p (w d) -> p w d", w=W),
                in1=rt[:].rearrange("p (w o) -> p w o", o=1).broadcast_to([128, W, C]),
                op=OP.mult,
            )
            nc.sync.dma_start(out=tv[cc], in_=ot[:])

    # -------- helper: one gather+reduce pass --------
    def hop_pass(tc, pool, spool, psum, d, idx_par, ldst_par, tab, iota):
        tabq = tab.rearrange("(q f) d -> q (f d)", f=4)
        isb = spool.tile([128, d["S"] // 16], i16, tag="hop_idx")
        lsb = spool.tile([128, d["NV"]], f32, tag="hop_ldst")
        nc.sync.dma_start(out=isb[:], in_=idx_par[:])
        nc.sync.dma_start(out=lsb[:], in_=ldst_par[:])
        acc = psum.tile([128, W * C], f32, space="PSUM")
        ncalls = d["S"] // CALL
        per = CALL // 128
        chmap = {}
        for e in d["chunks"]:
            chmap.setdefault(e[0] // per, []).append(e)
        for call in range(ncalls):
            gt = pool.tile([128, CALL // 128, 128], bf16, tag="hop_gt")
            nc.gpsimd.dma_gather(
                out_ap=gt[:],
                in_ap=tabq[:],
                idxs_ap=isb[:, call * (CALL // 16) : (call + 1) * (CALL // 16)],
                num_idxs=CALL,
                num_idxs_reg=CALL,
                elem_size=128,
            )
            for ch, wi, off, st, sp in chmap.get(call, []):
                j = ch % (CALL // 128)
                oh = pool.tile([128, 128], bf16, tag="hop_oh")
                nc.vector.tensor_scalar(
                    out=oh[:], in0=iota[:], scalar1=lsb[:, ch : ch + 1],
                    scalar2=None, op0=OP.is_equal,
                )
                nc.tensor.matmul(
                    acc[:, wi * C : (wi + 1) * C],
                    lhsT=oh[:],
                    rhs=gt[:, j, off : off + C],
                    start=st, stop=sp,
                )
        return acc

    # -------- helper: drain acc: raw f32 to dram, scaled bf16 to bounce ----
    def drain(tc, spool, acc, raw_dram, rpw_dram, bounce):
        tr = spool.tile([128, W * C], f32, tag="dr_raw")
        nc.vector.tensor_copy(out=tr[:], in_=acc[:])
        nc.sync.dma_start(out=raw_dram[:], in_=tr[:])
        if bounce is None:
            return
        rp = spool.tile([128, W], f32, tag="dr_rec")
        nc.sync.dma_start(out=rp[:], in_=rpw_dram[:])
        sc = spool.tile([128, W * C], bf16, tag="dr_sc")
        nc.vector.tensor_tensor(
            out=sc[:].rearrange("p (w d) -> p w d", w=W),
            in0=tr[:].rearrange("p (w d) -> p w d", w=W),
            in1=rp[:].rearrange("p (w o) -> p w o", o=1).broadcast_to([128, W, C]),
            op=OP.mult,
        )
        bv = bounce.rearrange("(w p) d -> p w d", p=128)
        nc.sync.dma_start(out=bv[:], in_=sc[:])

    def allgather(dst, srcb):
        return nc.gpsimd.collective_compute(
            "AllGather", OP.bypass, replica_groups=RG,
            ins=[srcb.ap().opt()], outs=[dst.ap().opt()],
        )

    # ================= CTX1: degrees =================
    with TC(nc) as tc:
        with (
            tc.tile_pool(name="p1", bufs=2) as pool,
            tc.tile_pool(name="ps1", bufs=1, space="PSUM") as psum,
            tc.tile_pool(name="c1", bufs=1) as cpool,
        ):
            iota = cpool.tile([128, 128], bf16)
            ident = cpool.tile([128, 128], f32)
            nc.sync.dma_start(out=iota[:], in_=io_bf[:])
            nc.sync.dma_start(out=ident[:], in_=id32[:])
            din, _ = deg_pass(tc, pool, cpool, psum, fwd, pin["fwgt"], pin["fldst"], iota)
            recip_export(tc, pool, psum, din, rpw_in, rin_flat, ident)
            dout, _ = deg_pass(tc, pool, cpool, psum, rev, pin["rwgt"], pin["rldst"], iota)
            recip_export(tc, pool, psum, dout, rpw_out, rout_flat, ident)

    with (
        nc.Block() as blk,
        nc.semaphore("cc1") as cc1,
    ):
        @blk.gpsimd
        def _(g):
            allgather(rin_full, rin_flat).then_inc(cc1, 1)
            g.wait_ge(cc1, 1)
            allgather(rout_full, rout_flat).then_inc(cc1, 1)
            g.wait_ge(cc1, 2)

    if stop_after == "ctx1":
        nc.compile()
        return nc

    # ================= CTX2: scaled x tables =================
    with TC(nc) as tc:
        with tc.tile_pool(name="p2", bufs=3) as pool:
            build_table(tc, pool, rout_full, xo_tab)
            build_table(tc, pool, rin_full, xi_tab)

    if stop_after == "ctx2":
        nc.compile()
        return nc

    # ================= CTX3: hop1 fwd =================
    with TC(nc) as tc:
        with (
            tc.tile_pool(name="p3", bufs=3) as pool,
            tc.tile_pool(name="ps3", bufs=1, space="PSUM") as psum,
            tc.tile_pool(name="c3", bufs=1) as cpool,
        ):
            iota = cpool.tile([128, 128], bf16)
            nc.sync.dma_start(out=iota[:], in_=io_bf[:])
            acc = hop_pass(tc, pool, cpool, psum, fwd, pin["fidx"], pin["fldst"], xo_tab, iota)
            drain(tc, cpool, acc, to1_raw, rpw_out, t1o_b)

    with (
        nc.Block() as blk2,
        nc.semaphore("cc2") as cc2,
    ):
        @blk2.gpsimd
        def _(g):
            allgather(t1o_tab, t1o_b).then_inc(cc2, 1)
            g.wait_ge(cc2, 1)

    if stop_after == "ctx3":
        nc.compile()
        return nc

    # ================= CTX4: hop1 rev =================
    with TC(nc) as tc:
        with (
            tc.tile_pool(name="p4", bufs=3) as pool,
            tc.tile_pool(name="ps4", bufs=1, space="PSUM") as psum,
            tc.tile_pool(name="c4", bufs=1) as cpool,
        ):
            iota = cpool.tile([128, 128], bf16)
            nc.sync.dma_start(out=iota[:], in_=io_bf[:])
            acc = hop_pass(tc, pool, cpool, psum, rev, pin["ridx"], pin["rldst"], xi_tab, iota)
            drain(tc, cpool, acc, ti1_raw, rpw_in, t1i_b)

    with (
        nc.Block() as blk3,
        nc.semaphore("cc3") as cc3,
    ):
        @blk3.gpsimd
        def _(g):
            allgather(t1i_tab, t1i_b).then_inc(cc3, 1)
            g.wait_ge(cc3, 1)

    if stop_after == "ctx4":
        nc.compile()
        return nc

    # ================= CTX5: hop2 fwd =================
    with TC(nc) as tc:
        with (
            tc.tile_pool(name="p5", bufs=3) as pool,
            tc.tile_pool(name="ps5", bufs=1, space="PSUM") as psum,
            tc.tile_pool(name="c5", bufs=1) as cpool,
        ):
            iota = cpool.tile([128, 128], bf16)
            nc.sync.dma_start(out=iota[:], in_=io_bf[:])
            acc = hop_pass(tc, pool, cpool, psum, fwd, pin["fidx"], pin["fldst"], t1o_tab, iota)
            drain(tc, cpool, acc, to2_raw, None, None)

    # ================= CTX6: hop2 rev + gates + output =================
    with TC(nc) as tc:
        with (
            tc.tile_pool(name="p6", bufs=3) as pool,
            tc.tile_pool(name="c6", bufs=1) as cpool,
        ):
            iota = cpool.tile([128, 128], bf16)
            ident = cpool.tile([128, 128], f32)
            nc.sync.dma_start(out=iota[:], in_=io_bf[:])
            nc.sync.dma_start(out=ident[:], in_=id32[:])
            ti2 = cpool.tile([128, W * C], f32)
            with tc.tile_pool(name="ps6", bufs=1, space="PSUM") as psum:
                acc = hop_pass(tc, pool, cpool, psum, rev, pin["ridx"], pin["rldst"], t1i_tab, iota)
                nc.vector.tensor_copy(out=ti2[:], in_=acc[:])
            psg_cm = tc.tile_pool(name="psg", bufs=2, space="PSUM")
            psg = psg_cm.__enter__()

            # F1 [128, sh]: rows 0:32 To1^T, 32:64 Ti1^T, 64:96 To2^T, 96:128 Ti2^T
            F1 = cpool.tile([128, sh], f32)
            for r, rawd in enumerate([to1_raw, ti1_raw, to2_raw]):
                tr = cpool.tile([128, W * C], f32, tag="ft_in")
                nc.sync.dma_start(out=tr[:], in_=rawd[:])
                for wi in range(W):
                    tp = psg.tile([C, 128], f32, space="PSUM", tag="ft_ps")
                    nc.tensor.transpose(
                        out=tp[:], in_=tr[:, wi * C : (wi + 1) * C], identity=ident[:]
                    )
                    nc.scalar.activation(
                        out=F1[r * C : (r + 1) * C, wi * 128 : (wi + 1) * 128],
                        in_=tp[:], func=AF.Copy,
                    )
            for wi in range(W):
                tp = psg.tile([C, 128], f32, space="PSUM", tag="ft_ps")
                nc.tensor.transpose(
                    out=tp[:], in_=ti2[:, wi * C : (wi + 1) * C], identity=ident[:]
                )
                nc.scalar.activation(
                    out=F1[3 * C : 4 * C, wi * 128 : (wi + 1) * 128], in_=tp[:], func=AF.Copy
                )

            # gate weights: W1 rows = [w(0,1), w(1,1), w(0,2), w(1,2)] blocks,
            # W2 = w(0,0)+w(1,0) (the x-term), matching F1 + streamed x^T
            W1 = cpool.tile([128, 128], f32)
            W2 = cpool.tile([C, 128], f32)
            wtmp = cpool.tile([C, 128], f32)
            for j in range(4):
                nc.sync.dma_start(out=W1[j * C : (j + 1) * C, :], in_=wstk[j + 2])
            nc.sync.dma_start(out=W2[:], in_=wstk[0])
            nc.sync.dma_start(out=wtmp[:], in_=wstk[1])
            nc.vector.tensor_tensor(out=W2[:], in0=W2[:], in1=wtmp[:], op=OP.add)
            nb = cpool.tile([128, 1], f32)
            nc.sync.dma_start(out=nb[:], in_=bcat[:])
            negb = cpool.tile([128, 1], f32)
            nc.vector.tensor_scalar(
                out=negb[:], in0=nb[:], scalar1=-1.0, scalar2=None, op0=OP.mult
            )
            lw = cpool.tile([GATE, OUTC], f32)
            lb = cpool.tile([OUTC, 1], f32)
            nc.sync.dma_start(out=lw[:], in_=linw[:])
            nc.sync.dma_start(out=lb[:], in_=linb[:])

            TILE = 512
            for t0 in range(0, sh, TILE):
                sl = slice(t0, t0 + TILE)
                xs = pool.tile([C, TILE], f32, tag="g_xs")
                nc.sync.dma_start(out=xs[:], in_=xT[:, sl])
                G = psg.tile([128, TILE], f32, space="PSUM", tag="g_ps")
                nc.tensor.matmul(G[:], lhsT=W1[:], rhs=F1[:, sl], start=True, stop=False)
                nc.tensor.matmul(G[:], lhsT=W2[:], rhs=xs[:], start=False, stop=True)
                zb = pool.tile([GATE, TILE], f32, tag="g_zb")
                ht = pool.tile([GATE, TILE], f32, tag="g_ht")
                nc.scalar.activation(
                    out=zb[:], in_=G[0:GATE, :], func=AF.Sigmoid,
                    bias=negb[0:GATE, :], scale=-1.0,
                )
                nc.scalar.activation(
                    out=ht[:], in_=G[GATE:128, :], func=AF.Tanh,
                    bias=nb[GATE:128, :], scale=1.0,
                )
                hs = pool.tile([GATE, TILE], f32, tag="g_hs")
                nc.vector.tensor_tensor(out=hs[:], in0=zb[:], in1=ht[:], op=OP.mult)
                hr = pool.tile([GATE, TILE], f32, tag="g_hr")
                nc.scalar.activation(out=hr[:], in_=hs[:], func=AF.Relu)
                po = psg.tile([OUTC, TILE], f32, space="PSUM", tag="o_ps")
                nc.tensor.matmul(po[:], lhsT=lw[:], rhs=hr[:], start=True, stop=True)
                ot = pool.tile([OUTC, TILE], f32, tag="g_ot")
                nc.vector.tensor_scalar(
                    out=ot[:], in0=po[:], scalar1=lb[:], scalar2=None, op0=OP.add
                )
                nc.sync.dma_start(out=outT[:, sl], in_=ot[:])
            psg_cm.__exit__(None, None, None)

    nc.compile()
    return nc


_CACHE = {}


def _get_built(x, edge_index, edge_weight):
    npad, sh, fwd, rev, x_pad = _host_prep(x, edge_index, edge_weight)
    nc = _build(npad, sh, fwd, rev)
    return npad, sh, fwd, rev, x_pad, nc


def kernel(x, edge_index, edge_weight, w_z, b_z, w_r, b_r, w_h, b_h, lin_w, lin_b):
    import ml_dtypes
    from concourse.bass_utils import run_bass_kernel_spmd

    x = np.asarray(x, np.float32)
    edge_index = np.asarray(edge_index)
    edge_weight = np.asarray(edge_weight, np.float32)
    import hashlib
    key = hashlib.sha1(
        np.ascontiguousarray(edge_index).tobytes()
        + np.ascontiguousarray(edge_weight).tobytes()
    ).hexdigest()
    if key not in _CACHE:
        _CACHE.clear()
        _CACHE[key] = _get_built(x, edge_index, edge_weight)
    npad, sh, fwd, rev, x_pad, nc = _CACHE[key]

    W = sh // 128
    iota = np.tile(np.arange(128, dtype=np.float32), (128, 1))
    wstk = np.zeros((6, 32, 128), np.float32)
    pairs = [(0, 0), (1, 0), (0, 1), (1, 1), (0, 2), (1, 2)]
    for j, (d, k) in enumerate(pairs):
        wstk[j, :, 0:64] = np.asarray(w_z, np.float32)[d, k, :32, :]
        wstk[j, :, 64:128] = np.asarray(w_h, np.float32)[d, k, :32, :]
    bcat = np.concatenate([np.asarray(b_z, np.float32), np.asarray(b_h, np.float32)])

    base = {
        "x_rm": x_pad,
        
        "io_bf": iota.astype(ml_dtypes.bfloat16),
        "id32": np.eye(128, dtype=np.float32),
        "wstk": wstk,
        "bcat": bcat.reshape(128, 1),
        "linw": np.asarray(lin_w, np.float32),
        "linb": np.asarray(lin_b, np.float32).reshape(OUTC, 1),
    }
    in_maps = []
    for c in range(NCORES):
        m = dict(base)
        m["xT"] = np.ascontiguousarray(x_pad.T[:, c * sh : (c + 1) * sh])
        m["fidx"] = fwd["gidx"][c]
        m["fldst"] = fwd["ldst"][c]
        m["fwgt"] = fwd["wgt"][c]
        m["ridx"] = rev["gidx"][c]
        m["rldst"] = rev["ldst"][c]
        m["rwgt"] = rev["wgt"][c]
        in_maps.append(m)

    import os
    trace = bool(int(os.environ.get("DCRNN_TRACE", "0")))
    res = run_bass_kernel_spmd(
        nc, in_maps, core_ids=list(range(NCORES)), trace=trace
    )
    global LAST_EXEC_NS
    LAST_EXEC_NS = res.exec_time_ns
    out = np.concatenate([res.results[c]["outT"] for c in range(NCORES)], axis=1)
    return np.ascontiguousarray(out.T[:N]).astype(np.float32)



# revision 17
# speedup vs baseline: 1.5615x; 1.5615x over previous
"""DCRNN diffusion-conv GNN forward on 8 trn2 NeuronCores.

Math (reference has H0=0, so the r-gate is dead and every dconv input is x):
  deg_out[v] = sum_{e:src=v} w[e]; deg_in[v] = sum_{e:dst=v} w[e]
  x_o = x / deg_out ; x_i = x / deg_in            (per-row scale)
  T_o1[d] = sum_{e:dst=d} x_o[src[e]]             (pure segment sums, coef
  T_i1[s] = sum_{e:src=s} x_i[dst[e]]              folded into the tables)
  T_o2 = segsum(T_o1/deg_out), T_i2 = segsum(T_i1/deg_in)
  G_g = x@(Wg[0,0]+Wg[1,0])[:32] + T_o1@Wg[0,1][:32] + T_i1@Wg[1,1][:32]
        + T_o2@Wg[0,2][:32] + T_i2@Wg[1,2][:32] + b_g      for g in {z,h}
  out = relu(sigmoid(-G_z) * tanh(G_h)) @ lin_w + lin_b

Distribution: nodes sharded 8 ways; edge lists partitioned by scatter-side
shard; gathers read replicated HBM tables (bf16, quad-row 256B descriptors);
segment sums are PE matmuls with DVE-built one-hot slot->node matrices into a
PSUM-resident [128, W*32] shard accumulator; shard T1 tables are exchanged
with AllGather. One SPMD program: per-(window,parity) slot budgets are maxed
across cores so the instruction stream is core-independent.
"""

import sys

sys.path.insert(0, "/opt/trn_rl_repo")

import numpy as np

N = 100000
C = 32
NCORES = 8
GATE = 64
OUTC = 32
CALL = 1024  # slots per dma_gather (2048+ risks SWDGE ring deadlock, 4096 confirmed hangs)


def _wrap_idx(a):
    # dma_gather index layout: idx i lives at partition i%16, col i//16,
    # replicated to all eight 16-partition groups.
    s = a.shape[0]
    w = a.reshape(s // 16, 16).T.astype(np.int16)
    return np.tile(w, (8, 1))


def _prep_dir(gnode, snode, wval, npad, sh):
    """Homogenized slot/chunk structure for one propagate direction.

    gnode: gather-side node per edge (global), snode: scatter-side node
    (global, defines core = snode//sh), wval: edge weight.
    Returns per-core device arrays + core-independent chunk metadata.
    """
    W = sh // 128
    core = snode // sh
    nl = snode - core * sh
    q = gnode % 4
    w = nl // 128
    cnt = np.zeros((NCORES, W, 4), np.int64)
    np.add.at(cnt, (core, w, q), 1)
    bud = 128 * np.ceil(cnt.max(axis=0) / 128).astype(np.int64)  # [W, 4]
    starts = np.concatenate([[0], np.cumsum(bud.reshape(-1))])[:-1].reshape(W, 4)
    S = int(bud.sum())
    S_pad = ((S + CALL - 1) // CALL) * CALL
    NV = S_pad // 128

    # slot position of each edge: starts[w,q] + rank within (core,w,q)
    key = (core * W + w) * 4 + q
    order = np.argsort(key, kind="stable")
    ranks = np.empty(len(key), np.int64)
    sk = key[order]
    brk = np.concatenate([[0], np.nonzero(np.diff(sk))[0] + 1])
    grp = np.zeros(len(sk), np.int64)
    grp[brk] = brk
    grp = np.maximum.accumulate(grp)
    ranks[order] = np.arange(len(sk)) - grp
    pos = starts[w, q] + ranks

    gidx_all, ldst_all, wgt_all = [], [], []
    for c in range(NCORES):
        m = core == c
        gidx = np.zeros(S_pad, np.int64)
        ldst = np.full(S_pad, -1.0, np.float32)
        wgt = np.zeros(S_pad, np.float32)
        gidx[pos[m]] = gnode[m] // 4
        ldst[pos[m]] = (nl[m] % 128).astype(np.float32)
        wgt[pos[m]] = wval[m]
        gidx_all.append(_wrap_idx(gidx))
        ldst_all.append(ldst.reshape(NV, 128).T.copy())
        wgt_all.append(wgt.reshape(NV, 128).T.copy())

    # chunk metadata (identical for every core)
    chunks = []  # (chunk_index, window, rhs_off, start, stop)
    for wi in range(W):
        cell = [(qi, k) for qi in range(4) for k in range(int(bud[wi, qi]) // 128)]
        for j, (qi, k) in enumerate(cell):
            ch = int(starts[wi, qi]) // 128 + k
            chunks.append((ch, wi, qi * 32, j == 0, j == len(cell) - 1))
    chunks.sort()
    return dict(
        S=S_pad, NV=NV, chunks=chunks,
        gidx=gidx_all, ldst=ldst_all, wgt=wgt_all,
    )


def _deg_stream(snode, wval, npad, sh):
    """Per-node weight stream for the degree reduce: [NCORES][128, W*B] f32.

    Element (core c, partition p, col w*B+b) = weight of the b-th edge whose
    scatter node is c*sh + 128*w + p (0-padded to the global max count B).
    """
    W = sh // 128
    cnt = np.bincount(snode, minlength=npad)
    B = max(int(cnt.max()), 1)
    order = np.argsort(snode, kind="stable")
    sk = snode[order]
    brk = np.concatenate([[0], np.nonzero(np.diff(sk))[0] + 1])
    grp = np.zeros(len(sk), np.int64)
    grp[brk] = brk
    grp = np.maximum.accumulate(grp)
    ranks = np.arange(len(sk)) - grp
    flat = np.zeros(npad * B, np.float32)
    flat[sk * B + ranks] = wval[order]
    per_core = []
    for c in range(NCORES):
        blk = flat[c * sh * B:(c + 1) * sh * B].reshape(W, 128, B)
        per_core.append(np.ascontiguousarray(blk.transpose(1, 0, 2).reshape(128, W * B)))
    return B, per_core


def _host_prep(x, edge_index, edge_weight):
    npad = ((N + 1024 * NCORES - 1) // (1024 * NCORES)) * 1024 * NCORES
    sh = npad // NCORES
    src = edge_index[0].astype(np.int64)
    dst = edge_index[1].astype(np.int64)
    wv = edge_weight.astype(np.float32)
    fwd = _prep_dir(src, dst, wv, npad, sh)  # scatter by dst, gather src
    rev = _prep_dir(dst, src, wv, npad, sh)  # scatter by src, gather dst
    fwd["degB"], fwd["degw"] = _deg_stream(dst, wv, npad, sh)  # deg_in
    rev["degB"], rev["degw"] = _deg_stream(src, wv, npad, sh)  # deg_out
    x_pad = np.zeros((npad, C), np.float32)
    x_pad[:N] = x
    return npad, sh, fwd, rev, x_pad


def _build(npad, sh, fwd, rev, stop_after=None):
    import concourse.bacc as bacc
    import concourse.bass as bass
    import concourse.mybir as mybir
    import concourse.tile as tile

    W = sh // 128
    f32 = mybir.dt.float32
    bf16 = mybir.dt.bfloat16
    i16 = mybir.dt.int16
    AF = mybir.ActivationFunctionType
    OP = mybir.AluOpType
    RG = [list(range(NCORES))]

    nc = bacc.Bacc(target_bir_lowering=False)

    # ---------------- parameters ----------------
    x_rm = nc.declare_dram_parameter("x_rm", [npad, C], f32, isOutput=False)
    xT = nc.declare_dram_parameter("xT", [C, sh], f32, isOutput=False)
    io_bf = nc.declare_dram_parameter("io_bf", [128, 128], bf16, isOutput=False)
    id32 = nc.declare_dram_parameter("id32", [128, 128], f32, isOutput=False)
    wstk = nc.declare_dram_parameter("wstk", [6, 32, 128], f32, isOutput=False)
    bcat = nc.declare_dram_parameter("bcat", [128, 1], f32, isOutput=False)
    linw = nc.declare_dram_parameter("linw", [GATE, OUTC], f32, isOutput=False)
    linb = nc.declare_dram_parameter("linb", [OUTC, 1], f32, isOutput=False)
    pin = {}
    for nm, d in (("f", fwd), ("r", rev)):
        pin[nm + "idx"] = nc.declare_dram_parameter(f"{nm}idx", [128, d["S"] // 16], i16, isOutput=False)
        pin[nm + "ldst"] = nc.declare_dram_parameter(f"{nm}ldst", [128, d["NV"]], f32, isOutput=False)
        pin[nm + "degw"] = nc.declare_dram_parameter(
            f"{nm}degw", [128, (sh // 128) * d["degB"]], f32, isOutput=False)
    outT = nc.declare_dram_parameter("outT", [C, sh], f32, isOutput=True)

    # ---------------- internal DRAM ----------------
    def dram(name, shape, dt):
        return nc.dram_tensor(name, shape, dt)

    rb_flat = dram("rb_flat", [2 * sh], f32)
    rb_full = nc.dram_tensor("rb_full", [NCORES * 2 * sh], f32, addr_space="Shared")
    rpw_in = dram("rpw_in", [128, W], f32)
    rpw_out = dram("rpw_out", [128, W], f32)
    xo_tab = dram("xo_tab", [npad, C], bf16)
    xi_tab = dram("xi_tab", [npad, C], bf16)
    t1o_b = dram("t1o_b", [sh, C], bf16)
    t1i_b = dram("t1i_b", [sh, C], bf16)
    t1o_tab = nc.dram_tensor("t1o_tab", [npad, C], bf16, addr_space="Shared")
    t1i_tab = nc.dram_tensor("t1i_tab", [npad, C], bf16, addr_space="Shared")
    to1_raw = dram("to1_raw", [128, W * C], f32)
    ti1_raw = dram("ti1_raw", [128, W * C], f32)
    to2_raw = dram("to2_raw", [128, W * C], f32)

    TC = tile.TileContext

    # -------- helper: deg via strided reduce over host-ordered weights --------
    def deg_reduce(tc, pool, degw_par, B):
        win = pool.tile([128, W * B], f32, tag="deg_win")
        nc.sync.dma_start(out=win[:], in_=degw_par[:])
        dps = pool.tile([128, W], f32, tag="deg_dps")
        nc.vector.tensor_reduce(
            out=dps[:], in_=win[:].rearrange("p (w b) -> p w b", b=B),
            axis=mybir.AxisListType.X, op=OP.add,
        )
        return dps

    # -------- helper: recip + row-major export --------
    def recip_export(tc, pool, psum, dps, rpw_dram, rflat_view, ident):
        rec = pool.tile([128, W], f32, tag="rec")
        nc.vector.tensor_scalar(
            out=rec[:], in0=dps[:], scalar1=1e-20, scalar2=None, op0=OP.max,
        )
        nc.vector.reciprocal(out=rec[:], in_=rec[:])
        nc.sync.dma_start(out=rpw_dram[:], in_=rec[:])
        tp = psum.tile([W, 128], f32, space="PSUM")
        nc.tensor.transpose(out=tp[:], in_=rec[:], identity=ident[:])
        rm = pool.tile([W, 128], f32, tag="rm")
        nc.scalar.activation(out=rm[:], in_=tp[:], func=AF.Copy)
        nc.sync.dma_start(out=rflat_view, in_=rm[:])

    # -------- helper: scaled-table build (x * recip -> bf16 table) --------
    def build_table(tc, pool, kdir, tab):
        xv = x_rm.rearrange("(c p w) d -> c p (w d)", p=128, w=W)
        rv = rb_full.rearrange("(c k p w) -> (c k) p w", k=2, p=128, w=W)
        tv = tab.rearrange("(c p w) d -> c p (w d)", p=128, w=W)
        for cc in range(NCORES):
            xt = pool.tile([128, W * C], f32, tag="xs_x")
            rt = pool.tile([128, W], f32, tag="xs_r")
            ot = pool.tile([128, W * C], bf16, tag="xs_o")
            nc.sync.dma_start(out=xt[:], in_=xv[cc])
            nc.sync.dma_start(out=rt[:], in_=rv[cc * 2 + kdir])
            nc.vector.tensor_tensor(
                out=ot[:].rearrange("p (w d) -> p w d", w=W),
                in0=xt[:].rearrange("p (w d) -> p w d", w=W),
                in1=rt[:].rearrange("p (w o) -> p w o", o=1).broadcast_to([128, W, C]),
                op=OP.mult,
            )
            nc.sync.dma_start(out=tv[cc], in_=ot[:])

    # -------- helper: one gather+reduce pass --------
    def hop_pass(tc, pool, spool, psum, d, idx_par, ldst_par, tab, iota):
        tabq = tab.rearrange("(q f) d -> q (f d)", f=4)
        isb = spool.tile([128, d["S"] // 16], i16, tag="hop_idx")
        lsb = spool.tile([128, d["NV"]], f32, tag="hop_ldst")
        nc.sync.dma_start(out=isb[:], in_=idx_par[:])
        nc.sync.dma_start(out=lsb[:], in_=ldst_par[:])
        acc = psum.tile([128, W * C], f32, space="PSUM")
        ncalls = d["S"] // CALL
        per = CALL // 128
        chmap = {}
        for e in d["chunks"]:
            chmap.setdefault(e[0] // per, []).append(e)
        for call in range(ncalls):
            gt = pool.tile([128, CALL // 128, 128], bf16, tag="hop_gt")
            nc.gpsimd.dma_gather(
                out_ap=gt[:],
                in_ap=tabq[:],
                idxs_ap=isb[:, call * (CALL // 16) : (call + 1) * (CALL // 16)],
                num_idxs=CALL,
                num_idxs_reg=CALL,
                elem_size=128,
            )
            for ch, wi, off, st, sp in chmap.get(call, []):
                j = ch % (CALL // 128)
                oh = pool.tile([128, 128], bf16, tag="hop_oh")
                nc.vector.tensor_scalar(
                    out=oh[:], in0=iota[:], scalar1=lsb[:, ch : ch + 1],
                    scalar2=None, op0=OP.is_equal,
                )
                nc.tensor.matmul(
                    acc[:, wi * C : (wi + 1) * C],
                    lhsT=oh[:],
                    rhs=gt[:, j, off : off + C],
                    start=st, stop=sp,
                )
        return acc

    # -------- helper: drain acc: raw f32 to dram, scaled bf16 to bounce ----
    def drain(tc, spool, acc, raw_dram, rpw_dram, bounce):
        tr = spool.tile([128, W * C], f32, tag="dr_raw")
        nc.vector.tensor_copy(out=tr[:], in_=acc[:])
        nc.sync.dma_start(out=raw_dram[:], in_=tr[:])
        if bounce is None:
            return
        rp = spool.tile([128, W], f32, tag="dr_rec")
        nc.sync.dma_start(out=rp[:], in_=rpw_dram[:])
        sc = spool.tile([128, W * C], bf16, tag="dr_sc")
        nc.vector.tensor_tensor(
            out=sc[:].rearrange("p (w d) -> p w d", w=W),
            in0=tr[:].rearrange("p (w d) -> p w d", w=W),
            in1=rp[:].rearrange("p (w o) -> p w o", o=1).broadcast_to([128, W, C]),
            op=OP.mult,
        )
        bv = bounce.rearrange("(w p) d -> p w d", p=128)
        nc.sync.dma_start(out=bv[:], in_=sc[:])

    def allgather(dst, srcb):
        return nc.gpsimd.collective_compute(
            "AllGather", OP.bypass, replica_groups=RG,
            ins=[srcb.ap().opt()], outs=[dst.ap().opt()],
        )

    # ================= CTX1: degrees =================
    rbv = rb_flat.rearrange("(k w p) -> k w p", k=2, p=128)
    with TC(nc) as tc:
        with (
            tc.tile_pool(name="p1", bufs=2) as pool,
            tc.tile_pool(name="ps1", bufs=1, space="PSUM") as psum,
            tc.tile_pool(name="c1", bufs=1) as cpool,
        ):
            ident = cpool.tile([128, 128], f32)
            nc.sync.dma_start(out=ident[:], in_=id32[:])
            din = deg_reduce(tc, pool, pin["fdegw"], fwd["degB"])
            recip_export(tc, pool, psum, din, rpw_in, rbv[0], ident)
            dout = deg_reduce(tc, pool, pin["rdegw"], rev["degB"])
            recip_export(tc, pool, psum, dout, rpw_out, rbv[1], ident)

    cc1 = nc.alloc_semaphore("cc1")
    cc2 = nc.alloc_semaphore("cc2")
    cc3 = nc.alloc_semaphore("cc3")
    with nc.Block() as blk:
        @blk.gpsimd
        def _(g):
            allgather(rb_full, rb_flat).then_inc(cc1, 1)
            g.wait_ge(cc1, 1)

    if stop_after == "ctx1":
        nc.compile()
        return nc

    # ================= CTX2: scaled x tables =================
    with TC(nc) as tc:
        with tc.tile_pool(name="p2", bufs=3) as pool:
            build_table(tc, pool, 1, xo_tab)
            build_table(tc, pool, 0, xi_tab)

    if stop_after == "ctx2":
        nc.compile()
        return nc

    # ================= CTX3: hop1 fwd =================
    with TC(nc) as tc:
        with (
            tc.tile_pool(name="p3", bufs=3) as pool,
            tc.tile_pool(name="ps3", bufs=1, space="PSUM") as psum,
            tc.tile_pool(name="c3", bufs=1) as cpool,
        ):
            iota = cpool.tile([128, 128], bf16)
            nc.sync.dma_start(out=iota[:], in_=io_bf[:])
            acc = hop_pass(tc, pool, cpool, psum, fwd, pin["fidx"], pin["fldst"], xo_tab, iota)
            drain(tc, cpool, acc, to1_raw, rpw_out, t1o_b)

    with nc.Block() as blk2:
        @blk2.gpsimd
        def _(g):
            allgather(t1o_tab, t1o_b).then_inc(cc2, 1)

    if stop_after == "ctx3":
        nc.compile()
        return nc

    # ================= CTX4: hop1 rev =================
    with TC(nc) as tc:
        with (
            tc.tile_pool(name="p4", bufs=3) as pool,
            tc.tile_pool(name="ps4", bufs=1, space="PSUM") as psum,
            tc.tile_pool(name="c4", bufs=1) as cpool,
        ):
            iota = cpool.tile([128, 128], bf16)
            nc.sync.dma_start(out=iota[:], in_=io_bf[:])
            acc = hop_pass(tc, pool, cpool, psum, rev, pin["ridx"], pin["rldst"], xi_tab, iota)
            drain(tc, cpool, acc, ti1_raw, rpw_in, t1i_b)

    with nc.Block() as blk3:
        @blk3.gpsimd
        def _(g):
            g.wait_ge(cc2, 1)
            allgather(t1i_tab, t1i_b).then_inc(cc3, 1)

    if stop_after == "ctx4":
        nc.compile()
        return nc

    # ================= CTX5: hop2 fwd =================
    with TC(nc) as tc:
        with (
            tc.tile_pool(name="p5", bufs=3) as pool,
            tc.tile_pool(name="ps5", bufs=1, space="PSUM") as psum,
            tc.tile_pool(name="c5", bufs=1) as cpool,
        ):
            iota = cpool.tile([128, 128], bf16)
            nc.sync.dma_start(out=iota[:], in_=io_bf[:])
            acc = hop_pass(tc, pool, cpool, psum, fwd, pin["fidx"], pin["fldst"], t1o_tab, iota)
            drain(tc, cpool, acc, to2_raw, None, None)

    with nc.Block() as blk4:
        @blk4.gpsimd
        def _(g):
            g.wait_ge(cc3, 1)

    # ================= CTX6: hop2 rev + gates + output =================
    with TC(nc) as tc:
        with (
            tc.tile_pool(name="p6", bufs=3) as pool,
            tc.tile_pool(name="c6", bufs=1) as cpool,
        ):
            iota = cpool.tile([128, 128], bf16)
            ident = cpool.tile([128, 128], f32)
            nc.sync.dma_start(out=iota[:], in_=io_bf[:])
            nc.sync.dma_start(out=ident[:], in_=id32[:])
            ti2 = cpool.tile([128, W * C], f32)
            with tc.tile_pool(name="ps6", bufs=1, space="PSUM") as psum:
                acc = hop_pass(tc, pool, cpool, psum, rev, pin["ridx"], pin["rldst"], t1i_tab, iota)
                nc.vector.tensor_copy(out=ti2[:], in_=acc[:])
            psg_cm = tc.tile_pool(name="psg", bufs=2, space="PSUM")
            psg = psg_cm.__enter__()

            # F1 [128, sh]: rows 0:32 To1^T, 32:64 Ti1^T, 64:96 To2^T, 96:128 Ti2^T
            F1 = cpool.tile([128, sh], f32)
            for r, rawd in enumerate([to1_raw, ti1_raw, to2_raw]):
                tr = cpool.tile([128, W * C], f32, tag="ft_in")
                nc.sync.dma_start(out=tr[:], in_=rawd[:])
                for wi in range(W):
                    tp = psg.tile([C, 128], f32, space="PSUM", tag="ft_ps")
                    nc.tensor.transpose(
                        out=tp[:], in_=tr[:, wi * C : (wi + 1) * C], identity=ident[:]
                    )
                    nc.scalar.activation(
                        out=F1[r * C : (r + 1) * C, wi * 128 : (wi + 1) * 128],
                        in_=tp[:], func=AF.Copy,
                    )
            for wi in range(W):
                tp = psg.tile([C, 128], f32, space="PSUM", tag="ft_ps")
                nc.tensor.transpose(
                    out=tp[:], in_=ti2[:, wi * C : (wi + 1) * C], identity=ident[:]
                )
                nc.scalar.activation(
                    out=F1[3 * C : 4 * C, wi * 128 : (wi + 1) * 128], in_=tp[:], func=AF.Copy
                )

            # gate weights: W1 rows = [w(0,1), w(1,1), w(0,2), w(1,2)] blocks,
            # W2 = w(0,0)+w(1,0) (the x-term), matching F1 + streamed x^T
            W1 = cpool.tile([128, 128], f32)
            W2 = cpool.tile([C, 128], f32)
            wtmp = cpool.tile([C, 128], f32)
            for j in range(4):
                nc.sync.dma_start(out=W1[j * C : (j + 1) * C, :], in_=wstk[j + 2])
            nc.sync.dma_start(out=W2[:], in_=wstk[0])
            nc.sync.dma_start(out=wtmp[:], in_=wstk[1])
            nc.vector.tensor_tensor(out=W2[:], in0=W2[:], in1=wtmp[:], op=OP.add)
            nb = cpool.tile([128, 1], f32)
            nc.sync.dma_start(out=nb[:], in_=bcat[:])
            negb = cpool.tile([128, 1], f32)
            nc.vector.tensor_scalar(
                out=negb[:], in0=nb[:], scalar1=-1.0, scalar2=None, op0=OP.mult
            )
            lw = cpool.tile([GATE, OUTC], f32)
            lb = cpool.tile([OUTC, 1], f32)
            nc.sync.dma_start(out=lw[:], in_=linw[:])
            nc.sync.dma_start(out=lb[:], in_=linb[:])

            TILE = 512
            for t0 in range(0, sh, TILE):
                sl = slice(t0, t0 + TILE)
                xs = pool.tile([C, TILE], f32, tag="g_xs")
                nc.sync.dma_start(out=xs[:], in_=xT[:, sl])
                G = psg.tile([128, TILE], f32, space="PSUM", tag="g_ps")
                nc.tensor.matmul(G[:], lhsT=W1[:], rhs=F1[:, sl], start=True, stop=False)
                nc.tensor.matmul(G[:], lhsT=W2[:], rhs=xs[:], start=False, stop=True)
                zb = pool.tile([GATE, TILE], f32, tag="g_zb")
                ht = pool.tile([GATE, TILE], f32, tag="g_ht")
                nc.scalar.activation(
                    out=zb[:], in_=G[0:GATE, :], func=AF.Sigmoid,
                    bias=negb[0:GATE, :], scale=-1.0,
                )
                nc.scalar.activation(
                    out=ht[:], in_=G[GATE:128, :], func=AF.Tanh,
                    bias=nb[GATE:128, :], scale=1.0,
                )
                hs = pool.tile([GATE, TILE], f32, tag="g_hs")
                nc.vector.tensor_tensor(out=hs[:], in0=zb[:], in1=ht[:], op=OP.mult)
                hr = pool.tile([GATE, TILE], f32, tag="g_hr")
                nc.scalar.activation(out=hr[:], in_=hs[:], func=AF.Relu)
                po = psg.tile([OUTC, TILE], f32, space="PSUM", tag="o_ps")
                nc.tensor.matmul(po[:], lhsT=lw[:], rhs=hr[:], start=True, stop=True)
                ot = pool.tile([OUTC, TILE], f32, tag="g_ot")
                nc.vector.tensor_scalar(
                    out=ot[:], in0=po[:], scalar1=lb[:], scalar2=None, op0=OP.add
                )
                nc.sync.dma_start(out=outT[:, sl], in_=ot[:])
            psg_cm.__exit__(None, None, None)

    nc.compile()
    return nc


_CACHE = {}


def _get_built(x, edge_index, edge_weight):
    npad, sh, fwd, rev, x_pad = _host_prep(x, edge_index, edge_weight)
    nc = _build(npad, sh, fwd, rev)
    return npad, sh, fwd, rev, x_pad, nc


def kernel(x, edge_index, edge_weight, w_z, b_z, w_r, b_r, w_h, b_h, lin_w, lin_b):
    import ml_dtypes
    from concourse.bass_utils import run_bass_kernel_spmd

    x = np.asarray(x, np.float32)
    edge_index = np.asarray(edge_index)
    edge_weight = np.asarray(edge_weight, np.float32)
    import hashlib
    key = hashlib.sha1(
        np.ascontiguousarray(edge_index).tobytes()
        + np.ascontiguousarray(edge_weight).tobytes()
    ).hexdigest()
    if key not in _CACHE:
        _CACHE.clear()
        _CACHE[key] = _get_built(x, edge_index, edge_weight)
    npad, sh, fwd, rev, x_pad, nc = _CACHE[key]

    W = sh // 128
    iota = np.tile(np.arange(128, dtype=np.float32), (128, 1))
    wstk = np.zeros((6, 32, 128), np.float32)
    pairs = [(0, 0), (1, 0), (0, 1), (1, 1), (0, 2), (1, 2)]
    for j, (d, k) in enumerate(pairs):
        wstk[j, :, 0:64] = np.asarray(w_z, np.float32)[d, k, :32, :]
        wstk[j, :, 64:128] = np.asarray(w_h, np.float32)[d, k, :32, :]
    bcat = np.concatenate([np.asarray(b_z, np.float32), np.asarray(b_h, np.float32)])

    base = {
        "x_rm": x_pad,
        
        "io_bf": iota.astype(ml_dtypes.bfloat16),
        "id32": np.eye(128, dtype=np.float32),
        "wstk": wstk,
        "bcat": bcat.reshape(128, 1),
        "linw": np.asarray(lin_w, np.float32),
        "linb": np.asarray(lin_b, np.float32).reshape(OUTC, 1),
    }
    in_maps = []
    for c in range(NCORES):
        m = dict(base)
        m["xT"] = np.ascontiguousarray(x_pad.T[:, c * sh : (c + 1) * sh])
        m["fidx"] = fwd["gidx"][c]
        m["fldst"] = fwd["ldst"][c]
        m["fdegw"] = fwd["degw"][c]
        m["ridx"] = rev["gidx"][c]
        m["rldst"] = rev["ldst"][c]
        m["rdegw"] = rev["degw"][c]
        in_maps.append(m)

    import os, tempfile
    trace = bool(int(os.environ.get("DCRNN_TRACE", "0")))
    tdir = None
    if trace and os.environ.get("DCRNN_TMPDIR"):
        tdir = tempfile.mkdtemp(dir=os.environ["DCRNN_TMPDIR"])
    res = run_bass_kernel_spmd(
        nc, in_maps, core_ids=list(range(NCORES)), trace=trace, tmpdir=tdir,
    )
    global LAST_EXEC_NS
    LAST_EXEC_NS = res.exec_time_ns
    out = np.concatenate([res.results[c]["outT"] for c in range(NCORES)], axis=1)
    return np.ascontiguousarray(out.T[:N]).astype(np.float32)

